# revision 1
# baseline (speedup 1.0000x reference)
"""Multi-head attention (B=2, S=2048, D=1024, H=16) on 8 TRN2 cores.

Sharding: core c -> batch b = c//4, head-group g = c%4 (heads 4g..4g+3,
projection dims 256g..256g+256). Each core computes a partial output
projection over its own 256 head-dims, then per-512-token-chunk 4-core
ReduceScatter(add) sums the partials and hands each core output dims
256r..256r+256; the collectives overlap later compute.

Device pipeline per core:
  1. q^T,k^T projections d-major [128, 2048] head-pair tiles; v
     projection token-major [2048, 4*68] with 4 aug columns per head
     (col 64+h is ones for head h -> per-head softmax denominator row).
  2. Head-outer software pipeline: per (sk tile, s4 half) a 2-bank QK
     matmul pair -> one fused exp over [128,1024]; AV accumulation into
     4 live [68,512] PSUM chunks lags one half-step so PE never waits
     on ACT. Denominators land on pso rows 64..67; summed into den_sb
     rows 0..3 (aligned 64-partition DVE shift).
  3. One reciprocal_approx_fast over [4,2048], selector matmuls
     broadcast per-(h,s4) recip rows to 64 partitions, in-place
     normalize at_sb; per s4: out-proj matmuls + ReduceScatter(add).
"""

import numpy as np
from contextlib import ExitStack

import concourse.bass as bass
import concourse.tile as tile
from concourse import mybir
from concourse._compat import with_exitstack

F32 = mybir.dt.float32
R32 = mybir.dt.float32r
AF = mybir.ActivationFunctionType


B, S, D = 2, 2048, 1024
NCORES, GROUP = 8, 4
DG = D // GROUP          # 256 projection dims per core
NH = 4                   # heads per core
DH = 64
SQ = 512                 # sq chunk (PSUM bank width in fp32)
NSQ = S // SQ            # 4
SKT = 128                # sk tile
NSK = S // SKT           # 16
KT = 128                 # contraction tile
NKT = D // KT            # 8
NAUG = 4                 # aug ones-columns per head (col 64+h hot)
VW = DH + NAUG           # 68 v_aug cols per head
SCALE = 0.125            # 1/sqrt(64)


@with_exitstack
def _mha(ctx: ExitStack, tc: "tile.TileContext", out, xq, xk, xv, wq, wk, wv, wo,
         maskb, sel, aug):
    nc = tc.nc
    P = 128

    # ---- persistent SBUF ----
    persist = ctx.enter_context(tc.tile_pool(name="persist", bufs=1))

    def T(shape, name, dt=F32):
        return persist.tile(shape, dt, name=name, tag=name)

    wq_sb = T([P, NKT * DG], "wq_sb", R32)
    wk_sb = T([P, NKT * DG], "wk_sb", R32)
    wv_sb = T([P, NKT * DG], "wv_sb", R32)
    wo_sb = T([P, 2 * D], "wo_sb", R32)
    mask_sb = T([P, NSK], "mask_sb")
    q_sb = T([P, 2 * S], "q_sb", R32)
    k_sb = T([P, 2 * S], "k_sb", R32)
    v_sb = T([P, NSK * NH * VW], "v_sb", R32)
    at_sb = T([P, 2 * S], "at_sb", R32)
    den_sb = T([NAUG, S], "den_sb")
    rec_f = T([NAUG, S], "rec_f")
    rec_r = T([NAUG, S], "rec_r", R32)
    sel_sb = T([NAUG, NH * DH], "sel_sb", R32)
    aug_sb = T([P, NH * NAUG], "aug_sb")
    nc.vector.memset(den_sb[:], 0.0)

    for k in range(NKT):
        nc.sync.dma_start(wq_sb[:, bass.ts(k, DG)], wq[bass.ts(k, P), :])
        nc.sync.dma_start(wk_sb[:, bass.ts(k, DG)], wk[bass.ts(k, P), :])
        nc.sync.dma_start(wv_sb[:, bass.ts(k, DG)], wv[bass.ts(k, P), :])
    for k in range(2):
        nc.sync.dma_start(wo_sb[:, bass.ts(k, D)], wo[bass.ts(k, P), :])
    nc.sync.dma_start(mask_sb[:], maskb[:, :])
    nc.sync.dma_start(sel_sb[:], sel[:, :])
    nc.sync.dma_start(aug_sb[:], aug[:, :])

    # ---- phase 1: projections ----
    with tc.tile_pool(name="xin", bufs=3) as xin_pool, \
         tc.tile_pool(name="ppqk", bufs=4, space="PSUM") as ppqk, \
         tc.tile_pool(name="ppv", bufs=2, space="PSUM") as ppv:
        for xdram, wsb, dst in ((xq, wq_sb, q_sb), (xk, wk_sb, k_sb)):
            for s4 in range(NSQ):
                xin = xin_pool.tile([P, NKT * SQ], R32, name="xin")
                for k in range(NKT):
                    nc.sync.dma_start(
                        xin[:, bass.ts(k, SQ)],
                        xdram[bass.ts(k, P), bass.ts(s4, SQ)],
                    )
                for d2 in range(2):
                    ps = ppqk.tile([P, SQ], F32, name="ps")
                    for k in range(NKT):
                        nc.tensor.matmul(
                            ps[:],
                            lhsT=wsb[:, bass.ds(k * DG + d2 * P, P)],
                            rhs=xin[:, bass.ts(k, SQ)],
                            start=(k == 0),
                            stop=(k == NKT - 1),
                        )
                    nc.vector.tensor_copy(
                        dst[:, bass.ds(d2 * S + s4 * SQ, SQ)], ps[:]
                    )

        for st in range(NSK):
            vin = xin_pool.tile([P, NKT * SKT], R32, name="vin")
            for k in range(NKT):
                nc.sync.dma_start(
                    vin[:, bass.ts(k, SKT)],
                    xv[bass.ts(k, P), bass.ts(st, SKT)],
                )
            psv = ppv.tile([P, DG], F32, name="psv")
            for k in range(NKT):
                nc.tensor.matmul(
                    psv[:],
                    lhsT=vin[:, bass.ts(k, SKT)],
                    rhs=wv_sb[:, bass.ts(k, DG)],
                    start=(k == 0),
                    stop=(k == NKT - 1),
                )
            base = st * NH * VW
            for h in range(NH):
                nc.vector.tensor_copy(
                    v_sb[:, bass.ds(base + h * VW, DH)], psv[:, bass.ts(h, DH)]
                )
                nc.vector.tensor_copy(
                    v_sb[:, bass.ds(base + h * VW + DH, NAUG)],
                    aug_sb[:, bass.ts(h, NAUG)],
                )

    # ---- phase 2: attention (h-outer, lag-1 AV pipeline) ----
    with tc.tile_pool(name="expp", bufs=3) as exp_pool, \
         tc.tile_pool(name="pslp", bufs=2, space="PSUM") as psl_pool, \
         tc.tile_pool(name="psop", bufs=1, space="PSUM") as pso_pool:
        for h in range(NH):
            pr, po = h // 2, (h % 2) * DH
            pso = [pso_pool.tile([VW, SQ], F32, name=f"pso{i}") for i in range(NSQ)]

            def emit_av(item):
                ex_t, sk_i, half_i = item
                for i in range(2):
                    s4 = half_i * 2 + i
                    nc.tensor.matmul(
                        pso[s4][:],
                        lhsT=v_sb[:, bass.ds(sk_i * NH * VW + h * VW, VW)],
                        rhs=ex_t[:, bass.ts(i, SQ)],
                        start=(sk_i == 0),
                        stop=(sk_i == NSK - 1),
                        skip_group_check=True,
                    )

            prev = None
            for sk in range(NSK):
                for half in range(2):
                    psl = psl_pool.tile([P, 2 * SQ], F32, name="psl")
                    for i in range(2):
                        s4 = half * 2 + i
                        nc.tensor.matmul(
                            psl[:, bass.ts(i, SQ)],
                            lhsT=k_sb[bass.ds(po, DH), bass.ds(pr * S + sk * SKT, SKT)],
                            rhs=q_sb[bass.ds(po, DH), bass.ds(pr * S + s4 * SQ, SQ)],
                            start=True,
                            stop=True,
                        )
                    ex = exp_pool.tile([P, 2 * SQ], R32, name="ex")
                    nc.scalar.activation(
                        ex[:],
                        psl[:],
                        AF.Exp,
                        bias=mask_sb[:, bass.ds(sk, 1)],
                        scale=SCALE,
                    )
                    if prev is not None:
                        emit_av(prev)
                    prev = (ex, sk, half)
            emit_av(prev)

            for s4 in range(NSQ):
                nc.vector.tensor_copy(
                    at_sb[bass.ds(po, DH), bass.ds(pr * S + s4 * SQ, SQ)],
                    pso[s4][bass.ds(0, DH), :],
                )
                nc.vector.tensor_add(
                    den_sb[:, bass.ts(s4, SQ)],
                    den_sb[:, bass.ts(s4, SQ)],
                    pso[s4][bass.ds(DH, NAUG), :],
                )

    # ---- normalize + phase 3: out-proj with per-chunk ReduceScatter ----
    nc.vector.reciprocal_approx_fast(rec_f[:], den_sb[:])
    nc.vector.tensor_copy(rec_r[:], rec_f[:])

    dram = ctx.enter_context(tc.tile_pool(name="dram", bufs=1, space="DRAM"))
    rs_in = [dram.tile([D, SQ], F32, name=f"rs_in{i}", tag=f"rs_in{i}")
             for i in range(NSQ)]
    rs_out = [dram.tile([DG, SQ], F32, name=f"rs_out{i}", tag=f"rs_out{i}")
              for i in range(NSQ)]

    with tc.tile_pool(name="psb", bufs=4, space="PSUM") as psb_pool, \
         tc.tile_pool(name="fin", bufs=2) as fin_pool, \
         tc.tile_pool(name="psf", bufs=2, space="PSUM") as psf_pool:
        for s4 in range(NSQ):
            for h in range(NH):
                pr, po = h // 2, (h % 2) * DH
                pb = psb_pool.tile([DH, SQ], F32, name="pb")
                nc.tensor.matmul(
                    pb[:],
                    lhsT=sel_sb[:, bass.ts(h, DH)],
                    rhs=rec_r[:, bass.ds(s4 * SQ, SQ)],
                    start=True,
                    stop=True,
                )
                nc.vector.tensor_mul(
                    at_sb[bass.ds(po, DH), bass.ds(pr * S + s4 * SQ, SQ)],
                    at_sb[bass.ds(po, DH), bass.ds(pr * S + s4 * SQ, SQ)],
                    pb[:],
                )

        for s4 in range(NSQ):
            for do8 in range(NKT):
                psf = psf_pool.tile([P, SQ], F32, name="psf")
                for kt in range(2):
                    nc.tensor.matmul(
                        psf[:],
                        lhsT=wo_sb[:, bass.ds(kt * D + do8 * P, P)],
                        rhs=at_sb[:, bass.ds(kt * S + s4 * SQ, SQ)],
                        start=(kt == 0),
                        stop=(kt == 1),
                    )
                ot = fin_pool.tile([P, SQ], F32, name="ot")
                nc.scalar.activation(ot[:], psf[:], AF.Copy)
                nc.sync.dma_start(rs_in[s4][bass.ts(do8, P), :], ot[:])
            nc.gpsimd.collective_compute(
                "ReduceScatter",
                mybir.AluOpType.add,
                replica_groups=[[0, 1, 2, 3], [4, 5, 6, 7]],
                ins=[rs_in[s4].opt()],
                outs=[rs_out[s4].opt()],
            )
            nc.sync.dma_start(out[:, bass.ts(s4, SQ)], rs_out[s4][:])


def build_program():
    from concourse import bacc

    nc = bacc.Bacc("TRN2", target_bir_lowering=False, debug=False, num_devices=NCORES)
    aps = {}
    for nm, shp, dt in (
        ("xq", [D, S], R32),
        ("xk", [D, S], R32),
        ("xv", [D, S], R32),
        ("wq", [D, DG], R32),
        ("wk", [D, DG], R32),
        ("wv", [D, DG], R32),
        ("wo", [DG, D], R32),
        ("maskb", [128, NSK], F32),
        ("sel", [NAUG, NH * DH], R32),
        ("aug", [128, NH * NAUG], F32),
    ):
        aps[nm] = nc.dram_tensor(nm, shp, dt, kind="ExternalInput").ap()
    out = nc.dram_tensor("out", [DG, S], F32, kind="ExternalOutput").ap()
    with tile.TileContext(nc) as tc:
        _mha(tc, out, **aps)
    nc.finalize()
    return nc


_NC_CACHE = None


def _get_program():
    global _NC_CACHE
    if _NC_CACHE is None:
        _NC_CACHE = build_program()
    return _NC_CACHE


def make_in_maps(query, key, value, mask, Wq, Wk, Wv, Wo):
    xT = {}
    for b in range(B):
        xT[("q", b)] = np.ascontiguousarray(query[b].T, dtype=np.float32)
        xT[("k", b)] = np.ascontiguousarray(key[b].T, dtype=np.float32)
        xT[("v", b)] = np.ascontiguousarray(value[b].T, dtype=np.float32)
    sel = np.zeros((NAUG, NH * DH), dtype=np.float32)
    aug = np.zeros((128, NH * NAUG), dtype=np.float32)
    for h in range(NH):
        sel[h, h * DH:(h + 1) * DH] = 1.0
        aug[:, h * NAUG + h] = 1.0
    in_maps = []
    for c in range(NCORES):
        b, g = divmod(c, GROUP)
        mrow = (mask[b].astype(np.float32) * np.float32(-1e9)).astype(np.float32)
        in_maps.append(
            {
                "xq": xT[("q", b)],
                "xk": xT[("k", b)],
                "xv": xT[("v", b)],
                "wq": np.ascontiguousarray(Wq[g * DG:(g + 1) * DG, :].T, dtype=np.float32),
                "wk": np.ascontiguousarray(Wk[g * DG:(g + 1) * DG, :].T, dtype=np.float32),
                "wv": np.ascontiguousarray(Wv[g * DG:(g + 1) * DG, :].T, dtype=np.float32),
                "wo": np.ascontiguousarray(Wo[:, g * DG:(g + 1) * DG].T, dtype=np.float32),
                "maskb": np.ascontiguousarray(mrow.reshape(NSK, 128).T),
                "sel": sel,
                "aug": aug,
            }
        )
    return in_maps


def assemble_output(results):
    out = np.empty((B, S, D), dtype=np.float32)
    for c in range(NCORES):
        b, r = divmod(c, GROUP)
        out[b, :, r * DG:(r + 1) * DG] = results[c]["out"].T
    return out


def kernel(query, key, value, mask, Wq, bq, Wk, bk, Wv, bv, Wo, bo, trace=False):
    from concourse.bass_utils import run_bass_kernel_spmd

    nc = _get_program()
    in_maps = make_in_maps(
        np.asarray(query), np.asarray(key), np.asarray(value), np.asarray(mask),
        np.asarray(Wq), np.asarray(Wk), np.asarray(Wv), np.asarray(Wo),
    )
    br = run_bass_kernel_spmd(nc, in_maps, list(range(NCORES)), trace=trace)
    out = assemble_output(br.results)
    if trace:
        return out, br
    return out



# revision 32
# speedup vs baseline: 1.3511x; 1.3511x over previous
"""Multi-head attention (B=2, S=2048, D=1024, H=16) on 8 TRN2 cores.

Sharding: core c -> batch b = c//4, head-group g = c%4 (heads 4g..4g+3,
projection dims 256g..256g+256). Each core computes a partial output
projection over its own 256 head-dims; per-512-token-chunk 4-core
ReduceScatter(add) in bf16 sums the partials, each core keeps dims
256r..256r+256; collectives overlap the next chunk's attention.

v2: bf16 compute everywhere (fp32 PSUM accumulation), ACT-saturated
attention pipeline:
  - projections q,k d-major [128, 2S] bf16; v token-major [128, NSK*256].
  - s4-outer attention: per (s4, pr, sk) one row-tiled QK matmul pair
    (head 2pr rows 0-63, head 2pr+1 rows 64-127 of the PE array) into
    psl [128, 1024]; ONE exp over [128,1024] (both heads) -> ex bf16;
    AV pair packed into pso [128, 512] (head a partitions 0-63, head b
    64-127) accumulating over sk. Softmax denominators accumulate on
    DVE (den_acc += ex) and reduce cross-partition via tiny ones-matmul
    column sums at each s4 boundary.
  - per-s4: reciprocal + selector-broadcast normalize, out-proj
    (wo bf16), bf16 ReduceScatter overlapped with next s4 attention.
"""

import numpy as np
from contextlib import ExitStack

import concourse.bass as bass
import concourse.tile as tile
from concourse import bass_isa, mybir
from concourse._compat import with_exitstack

F32 = mybir.dt.float32
R32 = mybir.dt.float32r
BF16 = mybir.dt.bfloat16
AF = mybir.ActivationFunctionType


B, S, D = 2, 2048, 1024
NCORES, GROUP = 8, 4
DG = D // GROUP          # 256 projection dims per core
NH = 4                   # heads per core
DH = 64
SQ = 512                 # sq chunk (PSUM bank width in fp32)
NSQ = S // SQ            # 4
SKT = 128                # sk tile
NSK = S // SKT           # 16
KT = 128                 # contraction tile
NKT = D // KT            # 8
SCALE = 0.125            # 1/sqrt(64)


@with_exitstack
def _mha(ctx: ExitStack, tc: "tile.TileContext", out, xq, xk, xv, wq, wk, wv, wo,
         maskb, dbg_at=None, dbg_den=None):
    nc = tc.nc
    P = 128

    # ---- persistent SBUF ----
    persist = ctx.enter_context(tc.tile_pool(name="persist", bufs=1))

    def T(shape, name, dt=F32):
        return persist.tile(shape, dt, name=name, tag=name)

    wq_sb = T([P, NKT * DG], "wq_sb", BF16)
    wk_sb = T([P, NKT * DG], "wk_sb", BF16)
    wv_sb = T([P, NKT * DG], "wv_sb", BF16)
    wo_sb = T([P, 2 * D], "wo_sb", BF16)
    mask_sb = T([P, NSK], "mask_sb")
    q_sb = T([P, 2 * S], "q_sb", BF16)
    k_sb = T([P, 2 * S], "k_sb", BF16)
    v_sb = T([P, NSK * NH * DH], "v_sb", BF16)
    at_sb = T([P, 2 * S], "at_sb", BF16)
    den_acc = T([P, 2 * 2 * SQ], "den_acc", R32)   # [128, h*512 + sq]
    den_red = T([P, 2 * 2 * SQ], "den_red")
    rec = T([P, 2 * 2 * SQ], "rec")

    for k in range(NKT):
        nc.sync.dma_start(wq_sb[:, bass.ts(k, DG)], wq[bass.ts(k, P), :])
        nc.sync.dma_start(wk_sb[:, bass.ts(k, DG)], wk[bass.ts(k, P), :])
        nc.sync.dma_start(wv_sb[:, bass.ts(k, DG)], wv[bass.ts(k, P), :])
    for k in range(2):
        nc.sync.dma_start(wo_sb[:, bass.ts(k, D)], wo[bass.ts(k, P), :])
    nc.sync.dma_start(mask_sb[:], maskb[:, :])

    # ---- phase 1: projections ----
    with tc.tile_pool(name="xin", bufs=3) as xin_pool, \
         tc.tile_pool(name="ppqk", bufs=4, space="PSUM") as ppqk, \
         tc.tile_pool(name="ppv", bufs=2, space="PSUM") as ppv:
        for xdram, wsb, dst in ((xk, wk_sb, k_sb), (xq, wq_sb, q_sb)):
            for s4 in range(NSQ):
                xin = xin_pool.tile([P, NKT * SQ], BF16, name="xin", tag="xin")
                for k in range(NKT):
                    nc.sync.dma_start(
                        xin[:, bass.ts(k, SQ)],
                        xdram[bass.ts(k, P), bass.ts(s4, SQ)],
                    )
                for pr in range(2):
                    ps = ppqk.tile([P, SQ], F32, name="ps")
                    for k in range(NKT):
                        nc.tensor.matmul(
                            ps[:],
                            lhsT=wsb[:, bass.ds(k * DG + pr * P, P)],
                            rhs=xin[:, bass.ts(k, SQ)],
                            start=(k == 0),
                            stop=(k == NKT - 1),
                        )
                    nc.vector.tensor_copy(
                        dst[:, bass.ds(pr * S + s4 * SQ, SQ)], ps[:]
                    )

        for st in range(NSK):
            vin = xin_pool.tile([P, NKT * SKT], BF16, name="vin", tag="xin")
            for k in range(NKT):
                nc.sync.dma_start(
                    vin[:, bass.ts(k, SKT)],
                    xv[bass.ts(k, P), bass.ts(st, SKT)],
                )
            psv = ppv.tile([P, DG], F32, name="psv")
            for k in range(NKT):
                nc.tensor.matmul(
                    psv[:],
                    lhsT=vin[:, bass.ts(k, SKT)],
                    rhs=wv_sb[:, bass.ts(k, DG)],
                    start=(k == 0),
                    stop=(k == NKT - 1),
                )
            nc.vector.tensor_copy(v_sb[:, bass.ts(st, DG)], psv[:])

    # ---- phases 2+3 fused: s4-outer attention + out-proj + ReduceScatter ----
    dram = ctx.enter_context(tc.tile_pool(name="dram", bufs=1, space="DRAM"))
    rs_in = [dram.tile([D, SQ], BF16, name=f"rs_in{i}", tag=f"rs_in{i}")
             for i in range(NSQ)]
    rs_out = [dram.tile([DG, SQ], BF16, name=f"rs_out{i}", tag=f"rs_out{i}")
              for i in range(NSQ)]

    with tc.tile_pool(name="pslp", bufs=2, space="PSUM") as psl_pool, \
         tc.tile_pool(name="psop", bufs=2, space="PSUM") as pso_pool, \
         tc.tile_pool(name="psfp", bufs=2, space="PSUM") as psf_pool, \
         tc.tile_pool(name="expp", bufs=3) as ex_pool, \
         tc.tile_pool(name="otp", bufs=2) as ot_pool:
        for s4 in range(NSQ):
            for pr in range(2):
                pso = pso_pool.tile([P, SQ], F32, name="pso")
                for sk in range(NSK):
                    psl = psl_pool.tile([P, 2 * SQ], F32, name="psl")
                    # QK: row-tiled pair (head a rows 0-63, head b rows 64-127)
                    for j in range(2):
                        nc.tensor.matmul(
                            psl[:, bass.ts(j, SQ)],
                            lhsT=k_sb[bass.ds(j * DH, DH),
                                      bass.ds(pr * S + sk * SKT, SKT)],
                            rhs=q_sb[bass.ds(j * DH, DH),
                                     bass.ds(pr * S + s4 * SQ, SQ)],
                            start=True,
                            stop=True,
                        )
                    ex = ex_pool.tile([P, 2 * SQ], BF16, name="ex")
                    nc.scalar.activation(
                        ex[:],
                        psl[:],
                        AF.Exp,
                        bias=mask_sb[:, bass.ds(sk, 1)],
                        scale=SCALE,
                    )
                    # denominator partials on DVE
                    dslc = den_acc[:, bass.ds(pr * 2 * SQ, 2 * SQ)]
                    if sk == 0:
                        nc.vector.tensor_copy(dslc, ex[:])
                    else:
                        nc.vector.tensor_add(dslc, dslc, ex[:])
                    # AV: col-packed pair into pso (head a parts 0-63, b 64-127)
                    for j in range(2):
                        nc.tensor.matmul(
                            pso[bass.ds(j * DH, DH), :],
                            lhsT=v_sb[:, bass.ds(sk * DG + (2 * pr + j) * DH, DH)],
                            rhs=ex[:, bass.ts(j, SQ)],
                            start=(sk == 0),
                            stop=(sk == NSK - 1),
                            skip_group_check=True,
                        )
                # drain unnormalized AV to at_sb (bf16)
                nc.vector.tensor_copy(
                    at_sb[:, bass.ds(pr * S + s4 * SQ, SQ)], pso[:]
                )

            # denominators: cross-partition all-reduce (gpsimd), reciprocal,
            # then partition-aligned normalize muls (recip of head 2pr+j is
            # broadcast on all partitions of rec's column block h)
            nc.gpsimd.partition_all_reduce(
                den_red[:], den_acc[:], channels=P,
                reduce_op=bass_isa.ReduceOp.add,
            )
            nc.vector.reciprocal_approx_fast(rec[:], den_red[:])
            for pr in range(2):
                for j in range(2):
                    nc.vector.tensor_mul(
                        at_sb[bass.ds(j * DH, DH), bass.ds(pr * S + s4 * SQ, SQ)],
                        at_sb[bass.ds(j * DH, DH), bass.ds(pr * S + s4 * SQ, SQ)],
                        rec[bass.ds(j * DH, DH), bass.ds((2 * pr + j) * SQ, SQ)],
                    )

            # out-projection for this s4 chunk + bf16 ReduceScatter
            for do8 in range(NKT):
                psf = psf_pool.tile([P, SQ], F32, name="psf")
                for pr in range(2):
                    nc.tensor.matmul(
                        psf[:],
                        lhsT=wo_sb[:, bass.ds(pr * D + do8 * P, P)],
                        rhs=at_sb[:, bass.ds(pr * S + s4 * SQ, SQ)],
                        start=(pr == 0),
                        stop=(pr == 1),
                    )
                ot = ot_pool.tile([P, SQ], BF16, name="ot")
                nc.vector.tensor_copy(ot[:], psf[:])
                nc.sync.dma_start(rs_in[s4][bass.ts(do8, P), :], ot[:])
            nc.gpsimd.collective_compute(
                "ReduceScatter",
                mybir.AluOpType.add,
                replica_groups=[[0, 1, 2, 3], [4, 5, 6, 7]],
                ins=[rs_in[s4].opt()],
                outs=[rs_out[s4].opt()],
            )
            nc.sync.dma_start(out[:, bass.ts(s4, SQ)], rs_out[s4][:])

    if dbg_at is not None:
        nc.sync.dma_start(dbg_at[:], at_sb[:])
        nc.sync.dma_start(dbg_den[:], den_red[:])


def build_program(debug=False):
    from concourse import bacc

    nc = bacc.Bacc("TRN2", target_bir_lowering=False, debug=False, num_devices=NCORES)
    aps = {}
    for nm, shp, dt in (
        ("xq", [D, S], BF16),
        ("xk", [D, S], BF16),
        ("xv", [D, S], BF16),
        ("wq", [D, DG], BF16),
        ("wk", [D, DG], BF16),
        ("wv", [D, DG], BF16),
        ("wo", [DG, D], BF16),
        ("maskb", [128, NSK], F32),
    ):
        aps[nm] = nc.dram_tensor(nm, shp, dt, kind="ExternalInput").ap()
    out = nc.dram_tensor("out", [DG, S], BF16, kind="ExternalOutput").ap()
    if debug:
        aps["dbg_at"] = nc.dram_tensor(
            "dbg_at", [128, 2 * S], BF16, kind="ExternalOutput").ap()
        aps["dbg_den"] = nc.dram_tensor(
            "dbg_den", [128, 2 * 2 * SQ], F32, kind="ExternalOutput").ap()
    with tile.TileContext(nc) as tc:
        _mha(tc, out, **aps)
    nc.finalize()
    return nc


_NC_CACHE = None


def _get_program():
    global _NC_CACHE
    if _NC_CACHE is None:
        _NC_CACHE = build_program()
    return _NC_CACHE


def make_in_maps(query, key, value, mask, Wq, Wk, Wv, Wo):
    import ml_dtypes

    bf = ml_dtypes.bfloat16
    xT = {}
    for b in range(B):
        xT[("q", b)] = np.ascontiguousarray(query[b].T).astype(bf)
        xT[("k", b)] = np.ascontiguousarray(key[b].T).astype(bf)
        xT[("v", b)] = np.ascontiguousarray(value[b].T).astype(bf)
    in_maps = []
    for c in range(NCORES):
        b, g = divmod(c, GROUP)
        mrow = (mask[b].astype(np.float32) * np.float32(-1e9)).astype(np.float32)
        in_maps.append(
            {
                "xq": xT[("q", b)],
                "xk": xT[("k", b)],
                "xv": xT[("v", b)],
                "wq": np.ascontiguousarray(Wq[g * DG:(g + 1) * DG, :].T).astype(bf),
                "wk": np.ascontiguousarray(Wk[g * DG:(g + 1) * DG, :].T).astype(bf),
                "wv": np.ascontiguousarray(Wv[g * DG:(g + 1) * DG, :].T).astype(bf),
                "wo": np.ascontiguousarray(Wo[:, g * DG:(g + 1) * DG].T).astype(bf),
                "maskb": np.ascontiguousarray(mrow.reshape(NSK, 128).T),
            }
        )
    return in_maps


def assemble_output(results):
    out = np.empty((B, S, D), dtype=np.float32)
    for c in range(NCORES):
        b, r = divmod(c, GROUP)
        out[b, :, r * DG:(r + 1) * DG] = results[c]["out"].T.astype(np.float32)
    return out


def kernel(query, key, value, mask, Wq, bq, Wk, bk, Wv, bv, Wo, bo, trace=False):
    from concourse.bass_utils import run_bass_kernel_spmd

    nc = _get_program()
    in_maps = make_in_maps(
        np.asarray(query), np.asarray(key), np.asarray(value), np.asarray(mask),
        np.asarray(Wq), np.asarray(Wk), np.asarray(Wv), np.asarray(Wo),
    )
    br = run_bass_kernel_spmd(nc, in_maps, list(range(NCORES)), trace=trace)
    out = assemble_output(br.results)
    if trace:
        return out, br
    return out


# revision 40
# speedup vs baseline: 1.5287x; 1.1315x over previous
"""Multi-head attention (B=2, S=2048, D=1024, H=16) on 8 TRN2 cores.

Sharding: core c -> batch b = c//4, head-group g = c%4 (heads 4g..4g+3,
projection dims 256g..256g+256). Each core computes a partial output
projection over its own 256 head-dims; per-512-token-chunk 4-core
ReduceScatter(add) in bf16 sums the partials, each core keeps dims
256r..256r+256; collectives overlap the next chunk's attention.

v2: bf16 compute everywhere (fp32 PSUM accumulation), ACT-saturated
attention pipeline:
  - projections q,k d-major [128, 2S] bf16; v token-major [128, NSK*256].
  - s4-outer attention: per (s4, pr, sk) one row-tiled QK matmul pair
    (head 2pr rows 0-63, head 2pr+1 rows 64-127 of the PE array) into
    psl [128, 1024]; ONE exp over [128,1024] (both heads) -> ex bf16;
    AV pair packed into pso [128, 512] (head a partitions 0-63, head b
    64-127) accumulating over sk. Softmax denominators accumulate on
    DVE (den_acc += ex) and reduce cross-partition via tiny ones-matmul
    column sums at each s4 boundary.
  - per-s4: reciprocal + selector-broadcast normalize, out-proj
    (wo bf16), bf16 ReduceScatter overlapped with next s4 attention.
"""

import numpy as np
from contextlib import ExitStack

import concourse.bass as bass
import concourse.tile as tile
from concourse import bass_isa, mybir
from concourse._compat import with_exitstack

F32 = mybir.dt.float32
R32 = mybir.dt.float32r
BF16 = mybir.dt.bfloat16
AF = mybir.ActivationFunctionType


B, S, D = 2, 2048, 1024
NCORES, GROUP = 8, 4
DG = D // GROUP          # 256 projection dims per core
NH = 4                   # heads per core
DH = 64
SQ = 512                 # sq chunk (PSUM bank width in fp32)
NSQ = S // SQ            # 4
SKT = 128                # sk tile
NSK = S // SKT           # 16
KT = 128                 # contraction tile
NKT = D // KT            # 8
SCALE = 0.125            # 1/sqrt(64)


@with_exitstack
def _mha(ctx: ExitStack, tc: "tile.TileContext", out, xq, xk, xv, wq, wk, wv, wo,
         maskb, aux, dbg_at=None):
    nc = tc.nc
    P = 128

    # ---- persistent SBUF ----
    persist = ctx.enter_context(tc.tile_pool(name="persist", bufs=1))

    def T(shape, name, dt=F32):
        return persist.tile(shape, dt, name=name, tag=name)

    wq_sb = T([P, NKT * DG], "wq_sb", BF16)
    wk_sb = T([P, NKT * DG], "wk_sb", BF16)
    wv_sb = T([P, NKT * DG], "wv_sb", BF16)
    wo_sb = T([P, 2 * D], "wo_sb", BF16)
    mask_sb = T([P, NSK], "mask_sb")
    q_sb = T([P, 2 * S], "q_sb", BF16)
    k_sb = T([P, 2 * S], "k_sb", BF16)
    v_sb = T([P, NSK * NH * DH], "v_sb", BF16)
    at_sb = T([P, 2 * S], "at_sb", BF16)
    rec = T([P, 2 * SQ], "rec")          # per-pr recip, pre-broadcast on parts
    ones_sb = T([P, DH], "ones_sb", BF16)

    for k in range(NKT):
        nc.sync.dma_start(wq_sb[:, bass.ts(k, DG)], wq[bass.ts(k, P), :])
        nc.sync.dma_start(wk_sb[:, bass.ts(k, DG)], wk[bass.ts(k, P), :])
        nc.sync.dma_start(wv_sb[:, bass.ts(k, DG)], wv[bass.ts(k, P), :])
    for k in range(2):
        nc.sync.dma_start(wo_sb[:, bass.ts(k, D)], wo[bass.ts(k, P), :])
    nc.sync.dma_start(mask_sb[:], maskb[:, :])
    nc.sync.dma_start(ones_sb[:], aux[:, :])

    # ---- phase 1: projections ----
    with tc.tile_pool(name="xin", bufs=3) as xin_pool, \
         tc.tile_pool(name="ppqk", bufs=4, space="PSUM") as ppqk, \
         tc.tile_pool(name="ppv", bufs=2, space="PSUM") as ppv:
        for xdram, wsb, dst in ((xk, wk_sb, k_sb), (xq, wq_sb, q_sb)):
            for s4 in range(NSQ):
                xin = xin_pool.tile([P, NKT * SQ], BF16, name="xin", tag="xin")
                for k in range(NKT):
                    nc.sync.dma_start(
                        xin[:, bass.ts(k, SQ)],
                        xdram[bass.ts(k, P), bass.ts(s4, SQ)],
                    )
                for pr in range(2):
                    ps = ppqk.tile([P, SQ], F32, name="ps")
                    for k in range(NKT):
                        nc.tensor.matmul(
                            ps[:],
                            lhsT=wsb[:, bass.ds(k * DG + pr * P, P)],
                            rhs=xin[:, bass.ts(k, SQ)],
                            start=(k == 0),
                            stop=(k == NKT - 1),
                        )
                    nc.vector.tensor_copy(
                        dst[:, bass.ds(pr * S + s4 * SQ, SQ)], ps[:]
                    )

        for st in range(NSK):
            vin = xin_pool.tile([P, NKT * SKT], BF16, name="vin", tag="xin")
            for k in range(NKT):
                nc.sync.dma_start(
                    vin[:, bass.ts(k, SKT)],
                    xv[bass.ts(k, P), bass.ts(st, SKT)],
                )
            psv = ppv.tile([P, DG], F32, name="psv")
            for k in range(NKT):
                nc.tensor.matmul(
                    psv[:],
                    lhsT=vin[:, bass.ts(k, SKT)],
                    rhs=wv_sb[:, bass.ts(k, DG)],
                    start=(k == 0),
                    stop=(k == NKT - 1),
                )
            nc.vector.tensor_copy(v_sb[:, bass.ts(st, DG)], psv[:])

    # ---- phases 2+3 fused: s4-outer attention + out-proj + ReduceScatter ----
    dram = ctx.enter_context(tc.tile_pool(name="dram", bufs=1, space="DRAM"))
    rs_in = [dram.tile([D, SQ], BF16, name=f"rs_in{i}", tag=f"rs_in{i}")
             for i in range(NSQ)]
    rs_out = [dram.tile([DG, SQ], BF16, name=f"rs_out{i}", tag=f"rs_out{i}")
              for i in range(NSQ)]

    with tc.tile_pool(name="pslp", bufs=2, space="PSUM") as psl_pool, \
         tc.tile_pool(name="psop", bufs=1, space="PSUM") as pso_pool, \
         tc.tile_pool(name="denp", bufs=2, space="PSUM") as den_pool, \
         tc.tile_pool(name="psfp", bufs=1, space="PSUM") as psf_pool, \
         tc.tile_pool(name="expp", bufs=4) as ex_pool, \
         tc.tile_pool(name="otp", bufs=2) as ot_pool:
        for s4 in range(NSQ):
            for pr in range(2):
                pso = pso_pool.tile([P, SQ], F32, name="pso")
                den_ps = den_pool.tile([P, SQ], F32, name="den_ps")
                for sk in range(NSK):
                    psl = psl_pool.tile([P, 2 * SQ], F32, name="psl")
                    # QK: row-tiled pair (head a rows 0-63, head b rows 64-127)
                    for j in range(2):
                        nc.tensor.matmul(
                            psl[:, bass.ts(j, SQ)],
                            lhsT=k_sb[bass.ds(j * DH, DH),
                                      bass.ds(pr * S + sk * SKT, SKT)],
                            rhs=q_sb[bass.ds(j * DH, DH),
                                     bass.ds(pr * S + s4 * SQ, SQ)],
                            start=True,
                            stop=True,
                        )
                    ex = ex_pool.tile([P, 2 * SQ], BF16, name="ex")
                    nc.scalar.activation(
                        ex[:],
                        psl[:],
                        AF.Exp,
                        bias=mask_sb[:, bass.ds(sk, 1)],
                        scale=SCALE,
                    )
                    # AV: col-packed pair into pso (head a parts 0-63, b 64-127)
                    # + denominator col-packed ones-matmul pair into den_ps
                    for j in range(2):
                        nc.tensor.matmul(
                            pso[bass.ds(j * DH, DH), :],
                            lhsT=v_sb[:, bass.ds(sk * DG + (2 * pr + j) * DH, DH)],
                            rhs=ex[:, bass.ts(j, SQ)],
                            start=(sk == 0),
                            stop=(sk == NSK - 1),
                            skip_group_check=True,
                        )
                    for j in range(2):
                        nc.tensor.matmul(
                            den_ps[bass.ds(j * DH, DH), :],
                            lhsT=ones_sb[:],
                            rhs=ex[:, bass.ts(j, SQ)],
                            start=(sk == 0),
                            stop=(sk == NSK - 1),
                            skip_group_check=True,
                        )
                # den_ps partitions j*64..j*64+63 all hold den(head 2pr+j):
                # reciprocal is already partition-broadcast; fuse drain+normalize
                nc.vector.reciprocal_approx_fast(
                    rec[:, bass.ts(pr, SQ)], den_ps[:]
                )
                nc.vector.tensor_mul(
                    at_sb[:, bass.ds(pr * S + s4 * SQ, SQ)],
                    pso[:],
                    rec[:, bass.ts(pr, SQ)],
                )

            # out-projection for this s4 chunk + bf16 ReduceScatter
            for do8 in range(NKT):
                psf = psf_pool.tile([P, SQ], F32, name="psf")
                for pr in range(2):
                    nc.tensor.matmul(
                        psf[:],
                        lhsT=wo_sb[:, bass.ds(pr * D + do8 * P, P)],
                        rhs=at_sb[:, bass.ds(pr * S + s4 * SQ, SQ)],
                        start=(pr == 0),
                        stop=(pr == 1),
                    )
                ot = ot_pool.tile([P, SQ], BF16, name="ot")
                nc.vector.tensor_copy(ot[:], psf[:])
                nc.sync.dma_start(rs_in[s4][bass.ts(do8, P), :], ot[:])
            nc.gpsimd.collective_compute(
                "ReduceScatter",
                mybir.AluOpType.add,
                replica_groups=[[0, 1, 2, 3], [4, 5, 6, 7]],
                ins=[rs_in[s4].opt()],
                outs=[rs_out[s4].opt()],
            )
            nc.sync.dma_start(out[:, bass.ts(s4, SQ)], rs_out[s4][:])

    if dbg_at is not None:
        nc.sync.dma_start(dbg_at[:], at_sb[:])


def build_program(debug=False):
    from concourse import bacc

    nc = bacc.Bacc("TRN2", target_bir_lowering=False, debug=False, num_devices=NCORES)
    aps = {}
    for nm, shp, dt in (
        ("xq", [D, S], BF16),
        ("xk", [D, S], BF16),
        ("xv", [D, S], BF16),
        ("wq", [D, DG], BF16),
        ("wk", [D, DG], BF16),
        ("wv", [D, DG], BF16),
        ("wo", [DG, D], BF16),
        ("maskb", [128, NSK], F32),
        ("aux", [128, DH], BF16),
    ):
        aps[nm] = nc.dram_tensor(nm, shp, dt, kind="ExternalInput").ap()
    out = nc.dram_tensor("out", [DG, S], BF16, kind="ExternalOutput").ap()
    if debug:
        aps["dbg_at"] = nc.dram_tensor(
            "dbg_at", [128, 2 * S], BF16, kind="ExternalOutput").ap()
    with tile.TileContext(nc) as tc:
        _mha(tc, out, **aps)
    nc.finalize()
    return nc


_NC_CACHE = None


def _get_program():
    global _NC_CACHE
    if _NC_CACHE is None:
        _NC_CACHE = build_program()
    return _NC_CACHE


def make_in_maps(query, key, value, mask, Wq, Wk, Wv, Wo):
    import ml_dtypes

    bf = ml_dtypes.bfloat16
    xT = {}
    for b in range(B):
        xT[("q", b)] = np.ascontiguousarray(query[b].T).astype(bf)
        xT[("k", b)] = np.ascontiguousarray(key[b].T).astype(bf)
        xT[("v", b)] = np.ascontiguousarray(value[b].T).astype(bf)
    in_maps = []
    for c in range(NCORES):
        b, g = divmod(c, GROUP)
        mrow = (mask[b].astype(np.float32) * np.float32(-1e9)).astype(np.float32)
        in_maps.append(
            {
                "xq": xT[("q", b)],
                "xk": xT[("k", b)],
                "xv": xT[("v", b)],
                "wq": np.ascontiguousarray(Wq[g * DG:(g + 1) * DG, :].T).astype(bf),
                "wk": np.ascontiguousarray(Wk[g * DG:(g + 1) * DG, :].T).astype(bf),
                "wv": np.ascontiguousarray(Wv[g * DG:(g + 1) * DG, :].T).astype(bf),
                "wo": np.ascontiguousarray(Wo[:, g * DG:(g + 1) * DG].T).astype(bf),
                "maskb": np.ascontiguousarray(mrow.reshape(NSK, 128).T),
                "aux": np.ones((128, DH), dtype=bf),
            }
        )
    return in_maps


def assemble_output(results):
    out = np.empty((B, S, D), dtype=np.float32)
    for c in range(NCORES):
        b, r = divmod(c, GROUP)
        out[b, :, r * DG:(r + 1) * DG] = results[c]["out"].T.astype(np.float32)
    return out


def kernel(query, key, value, mask, Wq, bq, Wk, bk, Wv, bv, Wo, bo, trace=False):
    from concourse.bass_utils import run_bass_kernel_spmd

    nc = _get_program()
    in_maps = make_in_maps(
        np.asarray(query), np.asarray(key), np.asarray(value), np.asarray(mask),
        np.asarray(Wq), np.asarray(Wk), np.asarray(Wv), np.asarray(Wo),
    )
    br = run_bass_kernel_spmd(nc, in_maps, list(range(NCORES)), trace=trace)
    out = assemble_output(br.results)
    if trace:
        return out, br
    return out


# revision 43
# speedup vs baseline: 1.6049x; 1.0498x over previous
"""Multi-head attention (B=2, S=2048, D=1024, H=16) on 8 TRN2 cores.

Sharding: core c -> batch b = c//4, head-group g = c%4 (heads 4g..4g+3,
projection dims 256g..256g+256). Each core computes a partial output
projection over its own 256 head-dims; per-512-token-chunk 4-core
ReduceScatter(add) in bf16 sums the partials, each core keeps dims
256r..256r+256; collectives overlap the next chunk's attention.

v2: bf16 compute everywhere (fp32 PSUM accumulation), ACT-saturated
attention pipeline:
  - projections q,k d-major [128, 2S] bf16; v token-major [128, NSK*256].
  - s4-outer attention: per (s4, pr, sk) one row-tiled QK matmul pair
    (head 2pr rows 0-63, head 2pr+1 rows 64-127 of the PE array) into
    psl [128, 1024]; ONE exp over [128,1024] (both heads) -> ex bf16;
    AV pair packed into pso [128, 512] (head a partitions 0-63, head b
    64-127) accumulating over sk. Softmax denominators accumulate on
    DVE (den_acc += ex) and reduce cross-partition via tiny ones-matmul
    column sums at each s4 boundary.
  - per-s4: reciprocal + selector-broadcast normalize, out-proj
    (wo bf16), bf16 ReduceScatter overlapped with next s4 attention.
"""

import numpy as np
from contextlib import ExitStack

import concourse.bass as bass
import concourse.tile as tile
from concourse import bass_isa, mybir
from concourse._compat import with_exitstack

F32 = mybir.dt.float32
R32 = mybir.dt.float32r
BF16 = mybir.dt.bfloat16
AF = mybir.ActivationFunctionType


B, S, D = 2, 2048, 1024
NCORES, GROUP = 8, 4
DG = D // GROUP          # 256 projection dims per core
NH = 4                   # heads per core
DH = 64
SQ = 512                 # sq chunk (PSUM bank width in fp32)
NSQ = S // SQ            # 4
SKT = 128                # sk tile
NSK = S // SKT           # 16
KT = 128                 # contraction tile
NKT = D // KT            # 8
SCALE = 0.125            # 1/sqrt(64)


@with_exitstack
def _mha(ctx: ExitStack, tc: "tile.TileContext", out, xq, xk, xv, wq, wk, wv, wo,
         maskb, aux, dbg_at=None):
    nc = tc.nc
    P = 128

    # ---- persistent SBUF ----
    persist = ctx.enter_context(tc.tile_pool(name="persist", bufs=1))

    def T(shape, name, dt=F32):
        return persist.tile(shape, dt, name=name, tag=name)

    wq_sb = T([P, NKT * DG], "wq_sb", BF16)
    wk_sb = T([P, NKT * DG], "wk_sb", BF16)
    wv_sb = T([P, NKT * DG], "wv_sb", BF16)
    wo_sb = T([P, 2 * D], "wo_sb", BF16)
    mask_sb = T([P, NSK], "mask_sb")
    q_sb = T([P, 2 * S], "q_sb", BF16)
    k_sb = T([P, 2 * S], "k_sb", BF16)
    v_sb = T([P, NSK * NH * DH], "v_sb", BF16)
    at_sb = T([P, 2 * S], "at_sb", BF16)
    rec = T([P, 2 * SQ], "rec")          # per-pr recip, pre-broadcast on parts
    ones_sb = T([P, DH], "ones_sb", BF16)

    for k in range(NKT):
        nc.sync.dma_start(wq_sb[:, bass.ts(k, DG)], wq[bass.ts(k, P), :])
        nc.sync.dma_start(wk_sb[:, bass.ts(k, DG)], wk[bass.ts(k, P), :])
        nc.sync.dma_start(wv_sb[:, bass.ts(k, DG)], wv[bass.ts(k, P), :])
    for k in range(2):
        nc.sync.dma_start(wo_sb[:, bass.ts(k, D)], wo[bass.ts(k, P), :])
    nc.sync.dma_start(mask_sb[:], maskb[:, :])
    nc.sync.dma_start(ones_sb[:], aux[:, :])

    # ---- phase 1: projections (kt-outer, full-row DMAs = 4KB lines) ----
    with tc.tile_pool(name="xin", bufs=2) as xin_pool, \
         tc.tile_pool(name="ppqk", bufs=1, space="PSUM") as ppqk:
        for xdram, wsb, dst in ((xk, wk_sb, k_sb), (xq, wq_sb, q_sb)):
            ps = [ppqk.tile([P, SQ], F32, name=f"ps{i}", tag=f"ps{i}")
                  for i in range(8)]
            for kt in range(NKT):
                xin = xin_pool.tile([P, S], BF16, name="xin", tag="xin")
                nc.sync.dma_start(xin[:], xdram[bass.ts(kt, P), :])
                for pr in range(2):
                    for s4 in range(NSQ):
                        nc.tensor.matmul(
                            ps[pr * NSQ + s4][:],
                            lhsT=wsb[:, bass.ds(kt * DG + pr * P, P)],
                            rhs=xin[:, bass.ts(s4, SQ)],
                            start=(kt == 0),
                            stop=(kt == NKT - 1),
                        )
            for pr in range(2):
                for s4 in range(NSQ):
                    nc.vector.tensor_copy(
                        dst[:, bass.ds(pr * S + s4 * SQ, SQ)],
                        ps[pr * NSQ + s4][:],
                    )

    with tc.tile_pool(name="vinp", bufs=2) as vin_pool, \
         tc.tile_pool(name="ppv", bufs=1, space="PSUM") as ppv:
        for r in range(2):
            psv = [ppv.tile([P, DG], F32, name=f"psv{i}", tag=f"psv{i}")
                   for i in range(8)]
            for kt in range(NKT):
                vin = vin_pool.tile([P, 8 * SKT], BF16, name="vin", tag="vin")
                nc.sync.dma_start(
                    vin[:], xv[bass.ts(kt, P), bass.ds(r * 8 * SKT, 8 * SKT)]
                )
                for st8 in range(8):
                    nc.tensor.matmul(
                        psv[st8][:],
                        lhsT=vin[:, bass.ts(st8, SKT)],
                        rhs=wv_sb[:, bass.ts(kt, DG)],
                        start=(kt == 0),
                        stop=(kt == NKT - 1),
                    )
            for st8 in range(8):
                nc.vector.tensor_copy(
                    v_sb[:, bass.ts(r * 8 + st8, DG)], psv[st8][:]
                )

    # ---- phases 2+3 fused: s4-outer attention + out-proj + ReduceScatter ----
    dram = ctx.enter_context(tc.tile_pool(name="dram", bufs=1, space="DRAM"))
    rs_in = [dram.tile([D, SQ], BF16, name=f"rs_in{i}", tag=f"rs_in{i}")
             for i in range(NSQ)]
    rs_out = [dram.tile([DG, SQ], BF16, name=f"rs_out{i}", tag=f"rs_out{i}")
              for i in range(NSQ)]

    with tc.tile_pool(name="pslp", bufs=2, space="PSUM") as psl_pool, \
         tc.tile_pool(name="psop", bufs=2, space="PSUM") as pso_pool, \
         tc.tile_pool(name="psfp", bufs=2, space="PSUM") as psf_pool, \
         tc.tile_pool(name="expp", bufs=4) as ex_pool, \
         tc.tile_pool(name="otp", bufs=2) as ot_pool:
        for s4 in range(NSQ):
            for pr in range(2):
                pso = pso_pool.tile([P, SQ], F32, name="pso", tag="pso")
                den_ps = pso_pool.tile([P, SQ], F32, name="den_ps", tag="pso")
                for sk in range(NSK):
                    psl = psl_pool.tile([P, 2 * SQ], F32, name="psl")
                    # QK: row-tiled pair (head a rows 0-63, head b rows 64-127)
                    for j in range(2):
                        nc.tensor.matmul(
                            psl[:, bass.ts(j, SQ)],
                            lhsT=k_sb[bass.ds(j * DH, DH),
                                      bass.ds(pr * S + sk * SKT, SKT)],
                            rhs=q_sb[bass.ds(j * DH, DH),
                                     bass.ds(pr * S + s4 * SQ, SQ)],
                            start=True,
                            stop=True,
                        )
                    ex = ex_pool.tile([P, 2 * SQ], BF16, name="ex")
                    nc.scalar.activation(
                        ex[:],
                        psl[:],
                        AF.Exp,
                        bias=mask_sb[:, bass.ds(sk, 1)],
                        scale=SCALE,
                    )
                    # AV: col-packed pair into pso (head a parts 0-63, b 64-127)
                    # + denominator col-packed ones-matmul pair into den_ps
                    for j in range(2):
                        nc.tensor.matmul(
                            pso[bass.ds(j * DH, DH), :],
                            lhsT=v_sb[:, bass.ds(sk * DG + (2 * pr + j) * DH, DH)],
                            rhs=ex[:, bass.ts(j, SQ)],
                            start=(sk == 0),
                            stop=(sk == NSK - 1),
                            skip_group_check=True,
                        )
                    for j in range(2):
                        nc.tensor.matmul(
                            den_ps[bass.ds(j * DH, DH), :],
                            lhsT=ones_sb[:],
                            rhs=ex[:, bass.ts(j, SQ)],
                            start=(sk == 0),
                            stop=(sk == NSK - 1),
                            skip_group_check=True,
                        )
                # den_ps partitions j*64..j*64+63 all hold den(head 2pr+j):
                # reciprocal is already partition-broadcast; fuse drain+normalize
                nc.vector.reciprocal_approx_fast(
                    rec[:, bass.ts(pr, SQ)], den_ps[:]
                )
                nc.vector.tensor_mul(
                    at_sb[:, bass.ds(pr * S + s4 * SQ, SQ)],
                    pso[:],
                    rec[:, bass.ts(pr, SQ)],
                )

            # out-projection for this s4 chunk + bf16 ReduceScatter
            for do8 in range(NKT):
                psf = psf_pool.tile([P, SQ], F32, name="psf")
                for pr in range(2):
                    nc.tensor.matmul(
                        psf[:],
                        lhsT=wo_sb[:, bass.ds(pr * D + do8 * P, P)],
                        rhs=at_sb[:, bass.ds(pr * S + s4 * SQ, SQ)],
                        start=(pr == 0),
                        stop=(pr == 1),
                    )
                ot = ot_pool.tile([P, SQ], BF16, name="ot")
                nc.vector.tensor_copy(ot[:], psf[:])
                nc.sync.dma_start(rs_in[s4][bass.ts(do8, P), :], ot[:])
            nc.gpsimd.collective_compute(
                "ReduceScatter",
                mybir.AluOpType.add,
                replica_groups=[[0, 1, 2, 3], [4, 5, 6, 7]],
                ins=[rs_in[s4].opt()],
                outs=[rs_out[s4].opt()],
            )
            nc.sync.dma_start(out[:, bass.ts(s4, SQ)], rs_out[s4][:])

    if dbg_at is not None:
        nc.sync.dma_start(dbg_at[:], at_sb[:])


def build_program(debug=False):
    from concourse import bacc

    nc = bacc.Bacc("TRN2", target_bir_lowering=False, debug=False, num_devices=NCORES)
    aps = {}
    for nm, shp, dt in (
        ("xq", [D, S], BF16),
        ("xk", [D, S], BF16),
        ("xv", [D, S], BF16),
        ("wq", [D, DG], BF16),
        ("wk", [D, DG], BF16),
        ("wv", [D, DG], BF16),
        ("wo", [DG, D], BF16),
        ("maskb", [128, NSK], F32),
        ("aux", [128, DH], BF16),
    ):
        aps[nm] = nc.dram_tensor(nm, shp, dt, kind="ExternalInput").ap()
    out = nc.dram_tensor("out", [DG, S], BF16, kind="ExternalOutput").ap()
    if debug:
        aps["dbg_at"] = nc.dram_tensor(
            "dbg_at", [128, 2 * S], BF16, kind="ExternalOutput").ap()
    with tile.TileContext(nc) as tc:
        _mha(tc, out, **aps)
    nc.finalize()
    return nc


_NC_CACHE = None


def _get_program():
    global _NC_CACHE
    if _NC_CACHE is None:
        _NC_CACHE = build_program()
    return _NC_CACHE


def make_in_maps(query, key, value, mask, Wq, Wk, Wv, Wo):
    import ml_dtypes

    bf = ml_dtypes.bfloat16
    xT = {}
    for b in range(B):
        xT[("q", b)] = np.ascontiguousarray(query[b].T).astype(bf)
        xT[("k", b)] = np.ascontiguousarray(key[b].T).astype(bf)
        xT[("v", b)] = np.ascontiguousarray(value[b].T).astype(bf)
    in_maps = []
    for c in range(NCORES):
        b, g = divmod(c, GROUP)
        mrow = (mask[b].astype(np.float32) * np.float32(-1e9)).astype(np.float32)
        in_maps.append(
            {
                "xq": xT[("q", b)],
                "xk": xT[("k", b)],
                "xv": xT[("v", b)],
                "wq": np.ascontiguousarray(Wq[g * DG:(g + 1) * DG, :].T).astype(bf),
                "wk": np.ascontiguousarray(Wk[g * DG:(g + 1) * DG, :].T).astype(bf),
                "wv": np.ascontiguousarray(Wv[g * DG:(g + 1) * DG, :].T).astype(bf),
                "wo": np.ascontiguousarray(Wo[:, g * DG:(g + 1) * DG].T).astype(bf),
                "maskb": np.ascontiguousarray(mrow.reshape(NSK, 128).T),
                "aux": np.ones((128, DH), dtype=bf),
            }
        )
    return in_maps


def assemble_output(results):
    out = np.empty((B, S, D), dtype=np.float32)
    for c in range(NCORES):
        b, r = divmod(c, GROUP)
        out[b, :, r * DG:(r + 1) * DG] = results[c]["out"].T.astype(np.float32)
    return out


def kernel(query, key, value, mask, Wq, bq, Wk, bk, Wv, bv, Wo, bo, trace=False):
    from concourse.bass_utils import run_bass_kernel_spmd

    nc = _get_program()
    in_maps = make_in_maps(
        np.asarray(query), np.asarray(key), np.asarray(value), np.asarray(mask),
        np.asarray(Wq), np.asarray(Wk), np.asarray(Wv), np.asarray(Wo),
    )
    br = run_bass_kernel_spmd(nc, in_maps, list(range(NCORES)), trace=trace)
    out = assemble_output(br.results)
    if trace:
        return out, br
    return out


# revision 48
# speedup vs baseline: 1.9245x; 1.1991x over previous
"""Multi-head attention (B=2, S=2048, D=1024, H=16) on 8 TRN2 cores.

Sharding: core c -> batch b = c//4, head-group g = c%4 (heads 4g..4g+3,
projection dims 256g..256g+256). Each core computes a partial output
projection over its own 256 head-dims; per-512-token-chunk 4-core
ReduceScatter(add) in bf16 sums the partials, each core keeps dims
256r..256r+256; collectives overlap the next chunk's attention.

v2: bf16 compute everywhere (fp32 PSUM accumulation), ACT-saturated
attention pipeline:
  - projections q,k d-major [128, 2S] bf16; v token-major [128, NSK*256].
  - s4-outer attention: per (s4, pr, sk) one row-tiled QK matmul pair
    (head 2pr rows 0-63, head 2pr+1 rows 64-127 of the PE array) into
    psl [128, 1024]; ONE exp over [128,1024] (both heads) -> ex bf16;
    AV pair packed into pso [128, 512] (head a partitions 0-63, head b
    64-127) accumulating over sk. Softmax denominators accumulate on
    DVE (den_acc += ex) and reduce cross-partition via tiny ones-matmul
    column sums at each s4 boundary.
  - per-s4: reciprocal + selector-broadcast normalize, out-proj
    (wo bf16), bf16 ReduceScatter overlapped with next s4 attention.
"""

import numpy as np
from contextlib import ExitStack

import concourse.bass as bass
import concourse.tile as tile
from concourse import bass_isa, mybir
from concourse._compat import with_exitstack

F32 = mybir.dt.float32
R32 = mybir.dt.float32r
BF16 = mybir.dt.bfloat16
AF = mybir.ActivationFunctionType


B, S, D = 2, 2048, 1024
NCORES, GROUP = 8, 4
DG = D // GROUP          # 256 projection dims per core
NH = 4                   # heads per core
DH = 64
SQ = 512                 # sq chunk (PSUM bank width in fp32)
NSQ = S // SQ            # 4
SKT = 128                # sk tile
NSK = S // SKT           # 16
KT = 128                 # contraction tile
NKT = D // KT            # 8
SCALE = 0.125            # 1/sqrt(64)


@with_exitstack
def _mha(ctx: ExitStack, tc: "tile.TileContext", out, xq, xk, xv, wq, wk, wv, wo,
         maskb, aux, dbg_at=None):
    nc = tc.nc
    P = 128

    # ---- persistent SBUF ----
    persist = ctx.enter_context(tc.tile_pool(name="persist", bufs=1))

    def T(shape, name, dt=F32):
        return persist.tile(shape, dt, name=name, tag=name)

    wq_sb = T([P, NKT * DG], "wq_sb", BF16)
    wk_sb = T([P, NKT * DG], "wk_sb", BF16)
    wv_sb = T([P, NKT * DG], "wv_sb", BF16)
    wo_sb = T([P, 2 * D], "wo_sb", BF16)
    mask_sb = T([P, NSK], "mask_sb")
    q_sb = T([P, 2 * S], "q_sb", BF16)
    k_sb = T([P, 2 * S], "k_sb", BF16)
    v_sb = T([P, NSK * NH * DH], "v_sb", BF16)
    at_sb = T([P, 2 * S], "at_sb", BF16)
    rec = T([P, 2 * SQ], "rec")          # per-pr recip, pre-broadcast on parts
    ones_sb = T([P, DH], "ones_sb", BF16)

    for k in range(NKT):
        nc.sync.dma_start(wq_sb[:, bass.ts(k, DG)], wq[bass.ts(k, P), :])
        nc.sync.dma_start(wk_sb[:, bass.ts(k, DG)], wk[bass.ts(k, P), :])
        nc.sync.dma_start(wv_sb[:, bass.ts(k, DG)], wv[bass.ts(k, P), :])
    for k in range(2):
        nc.sync.dma_start(wo_sb[:, bass.ts(k, D)], wo[bass.ts(k, P), :])
    nc.sync.dma_start(mask_sb[:], maskb[:, :])
    nc.sync.dma_start(ones_sb[:], aux[:, :])

    # ---- phase 1: projections (kt-outer, full-row DMAs = 4KB lines) ----
    with tc.tile_pool(name="xin", bufs=4) as xin_pool, \
         tc.tile_pool(name="ppqk", bufs=1, space="PSUM") as ppqk:
        for xdram, wsb, dst in ((xk, wk_sb, k_sb), (xq, wq_sb, q_sb)):
            ps = [ppqk.tile([P, SQ], F32, name=f"ps{i}", tag=f"ps{i}")
                  for i in range(8)]
            for kt in range(NKT):
                xin = xin_pool.tile([P, S], BF16, name="xin", tag="xin")
                for half in range(4):
                    nc.sync.dma_start(
                        xin[bass.ds(32 * half, 32), :],
                        xdram[bass.ds(kt * P + 32 * half, 32), :],
                    )
                for pr in range(2):
                    for s4 in range(NSQ):
                        nc.tensor.matmul(
                            ps[pr * NSQ + s4][:],
                            lhsT=wsb[:, bass.ds(kt * DG + pr * P, P)],
                            rhs=xin[:, bass.ts(s4, SQ)],
                            start=(kt == 0),
                            stop=(kt == NKT - 1),
                        )
            for pr in range(2):
                for s4 in range(NSQ):
                    nc.vector.tensor_copy(
                        dst[:, bass.ds(pr * S + s4 * SQ, SQ)],
                        ps[pr * NSQ + s4][:],
                    )

    with tc.tile_pool(name="vinp", bufs=4) as vin_pool, \
         tc.tile_pool(name="ppv", bufs=1, space="PSUM") as ppv:
        for r in range(2):
            psv = [ppv.tile([P, DG], F32, name=f"psv{i}", tag=f"psv{i}")
                   for i in range(8)]
            for kt in range(NKT):
                vin = vin_pool.tile([P, 8 * SKT], BF16, name="vin", tag="vin")
                for half in range(2):
                    nc.sync.dma_start(
                        vin[bass.ds(64 * half, 64), :],
                        xv[bass.ds(kt * P + 64 * half, 64),
                           bass.ds(r * 8 * SKT, 8 * SKT)],
                    )
                for st8 in range(8):
                    nc.tensor.matmul(
                        psv[st8][:],
                        lhsT=vin[:, bass.ts(st8, SKT)],
                        rhs=wv_sb[:, bass.ts(kt, DG)],
                        start=(kt == 0),
                        stop=(kt == NKT - 1),
                    )
            for st8 in range(8):
                nc.vector.tensor_copy(
                    v_sb[:, bass.ts(r * 8 + st8, DG)], psv[st8][:]
                )

    # ---- phases 2+3 fused: s4-outer attention + out-proj + ReduceScatter ----
    dram = ctx.enter_context(tc.tile_pool(name="dram", bufs=1, space="DRAM"))
    rs_in = [dram.tile([D, SQ], BF16, name=f"rs_in{i}", tag=f"rs_in{i}")
             for i in range(NSQ)]
    rs_out = [dram.tile([DG, SQ], BF16, name=f"rs_out{i}", tag=f"rs_out{i}")
              for i in range(NSQ)]

    with tc.tile_pool(name="pslp", bufs=3, space="PSUM") as psl_pool, \
         tc.tile_pool(name="psop", bufs=2, space="PSUM") as pso_pool, \
         tc.tile_pool(name="expp", bufs=4) as ex_pool, \
         tc.tile_pool(name="otp", bufs=2) as ot_pool:
        for s4 in range(NSQ):
            for pr in range(2):
                pso = pso_pool.tile([P, SQ], F32, name="pso", tag="pso")
                den_ps = pso_pool.tile([P, SQ], F32, name="den_ps", tag="pso")
                # sk processed in pairs: both QK matmul pairs emitted together
                # (one 64x128-mode stretch), then both exps, then AV+den
                # (one 128x64-mode stretch) — halves PE tiling-mode switches
                for skp in range(NSK // 2):
                    psls, exs = [], []
                    for sk in (2 * skp, 2 * skp + 1):
                        psl = psl_pool.tile([P, 2 * SQ], F32, name="psl",
                                            tag="psl")
                        psls.append(psl)
                        for j in range(2):
                            nc.tensor.matmul(
                                psl[:, bass.ts(j, SQ)],
                                lhsT=k_sb[bass.ds(j * DH, DH),
                                          bass.ds(pr * S + sk * SKT, SKT)],
                                rhs=q_sb[bass.ds(j * DH, DH),
                                         bass.ds(pr * S + s4 * SQ, SQ)],
                                start=True,
                                stop=True,
                            )
                    for i, sk in enumerate((2 * skp, 2 * skp + 1)):
                        ex = ex_pool.tile([P, 2 * SQ], BF16, name="ex",
                                          tag="ex")
                        exs.append(ex)
                        nc.scalar.activation(
                            ex[:],
                            psls[i][:],
                            AF.Exp,
                            bias=mask_sb[:, bass.ds(sk, 1)],
                            scale=SCALE,
                        )
                    for i, sk in enumerate((2 * skp, 2 * skp + 1)):
                        for j in range(2):
                            nc.tensor.matmul(
                                pso[bass.ds(j * DH, DH), :],
                                lhsT=v_sb[:, bass.ds(
                                    sk * DG + (2 * pr + j) * DH, DH)],
                                rhs=exs[i][:, bass.ts(j, SQ)],
                                start=(sk == 0),
                                stop=(sk == NSK - 1),
                                skip_group_check=True,
                            )
                        for j in range(2):
                            nc.tensor.matmul(
                                den_ps[bass.ds(j * DH, DH), :],
                                lhsT=ones_sb[:],
                                rhs=exs[i][:, bass.ts(j, SQ)],
                                start=(sk == 0),
                                stop=(sk == NSK - 1),
                                skip_group_check=True,
                            )
                # den_ps partitions j*64..j*64+63 all hold den(head 2pr+j):
                # reciprocal is already partition-broadcast; fuse drain+normalize
                nc.vector.reciprocal_approx_fast(
                    rec[:, bass.ts(pr, SQ)], den_ps[:]
                )
                nc.vector.tensor_mul(
                    at_sb[:, bass.ds(pr * S + s4 * SQ, SQ)],
                    pso[:],
                    rec[:, bass.ts(pr, SQ)],
                )

            # out-projection for this s4 chunk + bf16 ReduceScatter
            # (psf borrows the pso pool's two slots at the s4 boundary)
            for do8 in range(NKT):
                psf = pso_pool.tile([P, SQ], F32, name="psf", tag="pso")
                for pr in range(2):
                    nc.tensor.matmul(
                        psf[:],
                        lhsT=wo_sb[:, bass.ds(pr * D + do8 * P, P)],
                        rhs=at_sb[:, bass.ds(pr * S + s4 * SQ, SQ)],
                        start=(pr == 0),
                        stop=(pr == 1),
                    )
                ot = ot_pool.tile([P, SQ], BF16, name="ot")
                nc.vector.tensor_copy(ot[:], psf[:])
                nc.sync.dma_start(rs_in[s4][bass.ts(do8, P), :], ot[:])
            nc.gpsimd.collective_compute(
                "ReduceScatter",
                mybir.AluOpType.add,
                replica_groups=[[0, 1, 2, 3], [4, 5, 6, 7]],
                ins=[rs_in[s4].opt()],
                outs=[rs_out[s4].opt()],
            )
            nc.sync.dma_start(out[:, bass.ts(s4, SQ)], rs_out[s4][:])

    if dbg_at is not None:
        nc.sync.dma_start(dbg_at[:], at_sb[:])


def build_program(debug=False):
    from concourse import bacc

    nc = bacc.Bacc("TRN2", target_bir_lowering=False, debug=False, num_devices=NCORES)
    aps = {}
    for nm, shp, dt in (
        ("xq", [D, S], BF16),
        ("xk", [D, S], BF16),
        ("xv", [D, S], BF16),
        ("wq", [D, DG], BF16),
        ("wk", [D, DG], BF16),
        ("wv", [D, DG], BF16),
        ("wo", [DG, D], BF16),
        ("maskb", [128, NSK], F32),
        ("aux", [128, DH], BF16),
    ):
        aps[nm] = nc.dram_tensor(nm, shp, dt, kind="ExternalInput").ap()
    out = nc.dram_tensor("out", [DG, S], BF16, kind="ExternalOutput").ap()
    if debug:
        aps["dbg_at"] = nc.dram_tensor(
            "dbg_at", [128, 2 * S], BF16, kind="ExternalOutput").ap()
    with tile.TileContext(nc) as tc:
        _mha(tc, out, **aps)
    nc.finalize()
    return nc


_NC_CACHE = None


def _get_program():
    global _NC_CACHE
    if _NC_CACHE is None:
        _NC_CACHE = build_program()
    return _NC_CACHE


def make_in_maps(query, key, value, mask, Wq, Wk, Wv, Wo):
    import ml_dtypes

    bf = ml_dtypes.bfloat16
    xT = {}
    for b in range(B):
        xT[("q", b)] = np.ascontiguousarray(query[b].T).astype(bf)
        xT[("k", b)] = np.ascontiguousarray(key[b].T).astype(bf)
        xT[("v", b)] = np.ascontiguousarray(value[b].T).astype(bf)
    in_maps = []
    for c in range(NCORES):
        b, g = divmod(c, GROUP)
        mrow = (mask[b].astype(np.float32) * np.float32(-1e9)).astype(np.float32)
        in_maps.append(
            {
                "xq": xT[("q", b)],
                "xk": xT[("k", b)],
                "xv": xT[("v", b)],
                "wq": np.ascontiguousarray(Wq[g * DG:(g + 1) * DG, :].T).astype(bf),
                "wk": np.ascontiguousarray(Wk[g * DG:(g + 1) * DG, :].T).astype(bf),
                "wv": np.ascontiguousarray(Wv[g * DG:(g + 1) * DG, :].T).astype(bf),
                "wo": np.ascontiguousarray(Wo[:, g * DG:(g + 1) * DG].T).astype(bf),
                "maskb": np.ascontiguousarray(mrow.reshape(NSK, 128).T),
                "aux": np.ones((128, DH), dtype=bf),
            }
        )
    return in_maps


def assemble_output(results):
    out = np.empty((B, S, D), dtype=np.float32)
    for c in range(NCORES):
        b, r = divmod(c, GROUP)
        out[b, :, r * DG:(r + 1) * DG] = results[c]["out"].T.astype(np.float32)
    return out


def kernel(query, key, value, mask, Wq, bq, Wk, bk, Wv, bv, Wo, bo, trace=False):
    from concourse.bass_utils import run_bass_kernel_spmd

    nc = _get_program()
    in_maps = make_in_maps(
        np.asarray(query), np.asarray(key), np.asarray(value), np.asarray(mask),
        np.asarray(Wq), np.asarray(Wk), np.asarray(Wv), np.asarray(Wo),
    )
    br = run_bass_kernel_spmd(nc, in_maps, list(range(NCORES)), trace=trace)
    out = assemble_output(br.results)
    if trace:
        return out, br
    return out
